# revision 32
# baseline (speedup 1.0000x reference)
"""Trainium2 Bass kernel for nn_BasicNet (CondConv 3-branch + BN + channel shuffle).

v6 design (from HW profile of v3, 334us):
  - branch-major: per-branch BN-stat collectives overlap the next branch's
    convs (v3 had one collective at the end: 42us peer-wait + 70us dead tail).
  - sample-pair packing: each input tile holds two samples on the partition
    halves; per-tap matmuls use block-diagonal aggregated weights (K=128,
    M=128) so both samples' outputs land in one PSUM bank per tile. Plain
    matmuls only (no tile_position), single accumulation group per bank.
  - attention pooling on the PE: att_w/HW as a block-diagonal [128, 32]
    stationary operand, accumulated over free-dim chunks in PSUM; v3 burned
    25us of DVE tensor_reduce + 18us of ACT copies on pooled means.
  - PSUM evacuation at 128 partitions (both samples) with accum_out sums on
    ACT; sum-of-squares via DVE tensor_tensor_reduce on the stored bf16.
  - bf16 off-PE: weights + agg tree (DVE 2x/4x modes), normalize
    (tensor_scalar 4x), bf16 stores; host does upcast + channel shuffle +
    s0 passthrough (layout only).
"""

import sys

if '/opt/trn_rl_repo' not in sys.path:
    sys.path.insert(0, '/opt/trn_rl_repo')

import numpy as np
import ml_dtypes

import concourse.bass as bass
import concourse.bacc as bacc
import concourse.tile as tile
from concourse import mybir
from concourse import bass_utils

F32 = mybir.dt.float32
BF16 = mybir.dt.bfloat16

N_CORES = 8
NS = 4                   # samples per core
H = W = 56
HW = H * W               # 3136
C = 64                   # channels per branch (Cin == O == 64)
KEXP = 4                 # CondConv experts
RPT = 8                  # rows per conv tile
NT = RPT * W             # 448 free elements per matmul tile
N_TILES = H // RPT       # 7
M_TOTAL = 32 * HW        # BN stat count (global batch)
EPS = 1e-5
ROW_SLACK = 64           # zero tail per channel row
FOLD_CHUNK = 512
M_FOLD = 32              # fold matmul M (2 samples x 4 experts, zero-padded)

# branch geometry: (name, padded (ph,pw), taps)
BR = [
    ('sq', (58, 58), [(dy, dx) for dy in range(3) for dx in range(3)]),
    ('v', (58, 56), [(0, 0), (1, 0), (2, 0)]),
    ('h', (56, 58), [(0, 0), (0, 1), (0, 2)]),
]


def _build_nc():
    nc = bacc.Bacc('TRN2', target_bir_lowering=False, debug=False,
                   num_devices=N_CORES)

    xp = {}
    w_t = {}
    for bi, (bn, (ph, pw), taps) in enumerate(BR):
        xp[bi] = nc.dram_tensor(f'xp_{bn}', [NS, C, ph * pw + ROW_SLACK], BF16,
                                kind='ExternalInput').ap()
        w_t[bi] = nc.dram_tensor(f'w_{bn}', [128, KEXP, len(taps) * C], BF16,
                                 kind='ExternalInput').ap()
    att_fold = nc.dram_tensor('att_fold', [128, 3, M_FOLD], BF16,
                              kind='ExternalInput').ap()
    att_bias = nc.dram_tensor('att_bias', [M_FOLD, 3], F32,
                              kind='ExternalInput').ap()
    gb = nc.dram_tensor('gb', [128, 2, 3], F32, kind='ExternalInput').ap()
    out = nc.dram_tensor('out', [6, 128, HW], BF16,
                         kind='ExternalOutput').ap()

    with tile.TileContext(nc) as tc:
        _emit(tc, xp, w_t, att_fold, att_bias, gb, out)

    nc.compile()
    return nc


def _emit(tc, xp, w_t, att_fold, att_bias, gb, out):
    nc = tc.nc
    from contextlib import ExitStack
    ctx = ExitStack()
    with ctx:
        persist = ctx.enter_context(tc.tile_pool(name='persist', bufs=1))
        smalls = ctx.enter_context(tc.tile_pool(name='smalls', bufs=4))
        nrmp = ctx.enter_context(tc.tile_pool(name='nrmp', bufs=2))
        scrp = ctx.enter_context(tc.tile_pool(name='scrp', bufs=2))
        psum_conv = ctx.enter_context(
            tc.tile_pool(name='psum_conv', bufs=4, space='PSUM'))
        psum_att = ctx.enter_context(
            tc.tile_pool(name='psum_att', bufs=2, space='PSUM'))
        dram = ctx.enter_context(tc.tile_pool(name='dram', bufs=1, space='DRAM'))

        # ---------- ring-warmup dummy collective ----------
        cc_d_in = dram.tile([4, 1], F32, tag='cc_d_in')
        cc_d_out = dram.tile([4, 1], F32, tag='cc_d_out')
        nc.gpsimd.dma_start(out=cc_d_in, in_=att_bias[0:4, 0:1])
        nc.gpsimd.collective_compute(
            'AllReduce', mybir.AluOpType.add,
            replica_groups=[list(range(N_CORES))],
            ins=[cc_d_in.opt()], outs=[cc_d_out.opt()])

        # ---------- constants (SWDGE queue, ahead of the weights) ----------
        att_fold_sb = persist.tile([128, 3, M_FOLD], BF16, tag='att_fold_sb')
        nc.gpsimd.dma_start(out=att_fold_sb, in_=att_fold)
        att_bias_sb = persist.tile([M_FOLD, 3], F32, tag='att_bias_sb')
        nc.gpsimd.dma_start(out=att_bias_sb, in_=att_bias)
        gb_sb = persist.tile([128, 2, 3], F32, tag='gb_sb')
        nc.gpsimd.dma_start(out=gb_sb, in_=gb)
        epst = persist.tile([128, 1], F32, tag='epst')
        nc.vector.memset(epst, EPS)

        # ---------- input tiles: [sample 2p | sample 2p+1] ----------
        in_t = {}
        for bi, (bn, (ph, pw), taps) in enumerate(BR):
            flat = ph * pw
            for p in range(2):
                t = persist.tile([128, flat], BF16, tag=f'in_{bi}_{p}',
                                 name=f'in_{bi}_{p}')
                nc.sync.dma_start(out=t[0:64, :],
                                  in_=xp[bi][2 * p][:, 0:flat])
                nc.sync.dma_start(out=t[64:128, :],
                                  in_=xp[bi][2 * p + 1][:, 0:flat])
                in_t[(bi, p)] = t

        # ---------- weights (SWDGE queue, off the input path) ----------
        w_sb = {}
        for bi, (bn, _, taps) in enumerate(BR):
            t = persist.tile([128, KEXP, len(taps) * C], BF16,
                             tag=f'w_sb_{bi}', name=f'w_sb_{bi}')
            nc.gpsimd.dma_start(out=t, in_=w_t[bi])
            w_sb[bi] = t

        # block-diagonal aggregated weights, one per (branch, pair); the
        # off-diagonal halves are zeroed once and never rewritten
        ag_t = {}
        for bi, (bn, _, taps) in enumerate(BR):
            for p in range(2):
                t = persist.tile([128, len(taps) * 2 * C], BF16,
                                 tag=f'ag_{bi}_{p}', name=f'ag_{bi}_{p}')
                if bi == 0:
                    nc.gpsimd.memset(t, 0.0)
                ag_t[(bi, p)] = t

        # ---------- persistent outputs / stats ----------
        otiles = [persist.tile([128, HW], BF16, tag=f'ot_{i}', name=f'ot_{i}')
                  for i in range(6)]
        sums = [persist.tile([128, N_TILES], F32, tag=f'sums_{i}',
                             name=f'sums_{i}') for i in range(6)]
        sqs = [persist.tile([128, N_TILES], F32, tag=f'sqs_{i}',
                            name=f'sqs_{i}') for i in range(6)]

        cc_in_all = dram.tile([3, 2, 128, 2], F32, tag='cc_in_all')
        cc_out_all = dram.tile([3, 2, 128, 2], F32, tag='cc_out_all')
        cc_in = [cc_in_all[b] for b in range(3)]
        cc_out = [cc_out_all[b] for b in range(3)]

        # ---------- branches ----------
        # fold_att(b): PE pooled fold + sigmoid + DVE-transpose broadcast.
        # aggs(b): block-diagonal aggregated weights on DVE.
        # conv_pair(b, p): convs + evac + stats + accumulating cc staging.
        # consume(b): collective result -> scale/bias -> normalize -> store.
        # Emission interleaves these so folds hide between conv blocks and no
        # engine's program stalls on a collective while conv work remains.
        att_bcs = {}

        def fold_att(bi):
            bn, (ph, pw), taps = BR[bi]
            flat = ph * pw
            nch = (flat + FOLD_CHUNK - 1) // FOLD_CHUNK
            if bi > 0:
                for p in range(2):
                    nc.gpsimd.memset(ag_t[(bi, p)], 0.0)
            for p in range(2):
                att_ps = psum_att.tile([M_FOLD, FOLD_CHUNK], F32, tag='att_ps')
                for ci in range(nch):
                    c0 = ci * FOLD_CHUNK
                    c1 = min(flat, c0 + FOLD_CHUNK)
                    nc.tensor.matmul(
                        att_ps[:, 0:c1 - c0],
                        lhsT=att_fold_sb[:, bi, :],
                        rhs=in_t[(bi, p)][:, c0:c1],
                        start=(ci == 0), stop=(ci == nch - 1))
                att32 = smalls.tile([M_FOLD, M_FOLD], F32, tag='att32',
                                    name=f'att32_{bi}_{p}')
                nc.vector.memset(att32, 0.0)
                attv = smalls.tile([M_FOLD, 1], F32, tag='attv')
                nc.vector.tensor_reduce(out=attv, in_=att_ps,
                                        axis=mybir.AxisListType.X,
                                        op=mybir.AluOpType.add)
                nc.scalar.activation(out=att32[:, 0:1], in_=attv,
                                     func=mybir.ActivationFunctionType.Sigmoid,
                                     bias=att_bias_sb[:, bi:bi + 1])
                att_t = smalls.tile([M_FOLD, M_FOLD], F32, tag='att_t',
                                    name=f'att_t_{bi}_{p}')
                nc.vector.transpose(out=att_t, in_=att32)
                att_bc = smalls.tile([128, 2 * KEXP], F32, tag='att_bc',
                                     name=f'att_bc_{bi}_{p}')
                nc.gpsimd.partition_broadcast(att_bc, att_t[0:1, 0:2 * KEXP])
                att_bcs[(bi, p)] = att_bc

        def aggs(bi):
            # att_bc col for (half h, expert k) = 4h + k
            for p in range(2):
                ag = ag_t[(bi, p)]
                att_bc = att_bcs[(bi, p)]
                ag4 = ag.rearrange('p (t two o) -> p t two o', two=2, o=C)
                w4 = w_sb[bi].rearrange('p k (t o) -> p k t o', o=C)
                for h in range(2):
                    dst = ag4[64 * h:64 * h + 64, :, h, :]
                    for k in range(KEXP):
                        col = 4 * h + k
                        src = w4[64 * h:64 * h + 64, k]
                        if k == 0:
                            nc.vector.tensor_scalar_mul(
                                out=dst, in0=src,
                                scalar1=att_bc[64 * h:64 * h + 64,
                                               col:col + 1])
                        else:
                            nc.vector.scalar_tensor_tensor(
                                out=dst, in0=src,
                                scalar=att_bc[64 * h:64 * h + 64,
                                              col:col + 1],
                                in1=dst, op0=mybir.AluOpType.mult,
                                op1=mybir.AluOpType.add)

        def conv_pair(bi, p):
            bn, (ph, pw), taps = BR[bi]
            ntap = len(taps)
            oi = 2 * bi + p
            otile = otiles[oi]
            it3 = in_t[(bi, p)].rearrange('c (r q) -> c r q', q=pw)
            ag = ag_t[(bi, p)]
            for t in range(N_TILES):
                pt = psum_conv.tile([128, NT], F32, tag='pt')
                for ti, (dy, dx) in enumerate(taps):
                    r0 = RPT * t + dy
                    nc.tensor.matmul(
                        pt, lhsT=ag[:, ti * 2 * C:(ti + 1) * 2 * C],
                        rhs=it3[:, r0:r0 + RPT, dx:dx + W],
                        start=(ti == 0), stop=(ti == ntap - 1))
                # evacuate (bf16) + channel sums on ACT
                nc.scalar.activation(
                    out=otile[:, t * NT:(t + 1) * NT], in_=pt,
                    func=mybir.ActivationFunctionType.Copy,
                    accum_out=sums[oi][:, t:t + 1])
                # sum of squares on DVE from the stored bf16
                scr = scrp.tile([128, NT], BF16, tag='scr')
                osl = otile[:, t * NT:(t + 1) * NT]
                nc.vector.scalar_tensor_tensor(
                    out=scr, in0=osl, scalar=1.0, in1=osl,
                    op0=mybir.AluOpType.mult, op1=mybir.AluOpType.mult,
                    accum_out=sqs[oi][:, t:t + 1])
            # per-otile totals; the four partition-half blocks accumulate
            # into one [64, 2] cc buffer via DMA-compute (first overwrites)
            red = smalls.tile([128, 2], F32, tag='red', name=f'red_{oi}')
            nc.vector.tensor_reduce(out=red[:, 0:1], in_=sums[oi],
                                    axis=mybir.AxisListType.X,
                                    op=mybir.AluOpType.add)
            nc.vector.tensor_reduce(out=red[:, 1:2], in_=sqs[oi],
                                    axis=mybir.AxisListType.X,
                                    op=mybir.AluOpType.add)
            nc.sync.dma_start(out=cc_in[bi][p], in_=red)

        def collective_all():
            nc.gpsimd.collective_compute(
                'AllReduce', mybir.AluOpType.add,
                replica_groups=[list(range(N_CORES))],
                ins=[cc_in_all.opt()], outs=[cc_out_all.opt()])

        def consume_all(gate):
            # One fused post-collective phase for all three branches. The
            # zero-multiplied `gate` (the last sumsq stats) keeps the chain's
            # scheduler readiness behind the producer work so nothing here is
            # hoisted in front of it (the list scheduler thinks collectives
            # are fast and would otherwise stall engine queues on HW).
            zg = smalls.tile([128, 12], F32, tag='zg')
            nc.vector.tensor_scalar_mul(out=zg[:, 0:6], in0=gate[:, 1:7],
                                        scalar1=0.0)
            nc.vector.tensor_scalar_mul(out=zg[:, 6:12], in0=gate[:, 1:7],
                                        scalar1=0.0)
            # [128, 3(branch), 2(otile), 2(stat)] straight + halves swapped
            cr = cc_out_all.rearrange('b o p s -> p b o s')
            gA = smalls.tile([128, 3, 2, 2], F32, tag='gA')
            nc.sync.dma_start(out=gA, in_=cr)
            gB = smalls.tile([128, 3, 2, 2], F32, tag='gB')
            nc.gpsimd.dma_start(out=gB[0:64], in_=cr[64:128])
            nc.gpsimd.dma_start(out=gB[64:128], in_=cr[0:64])
            gAf = gA.rearrange('p b o s -> p (b o s)')
            gBf = gB.rearrange('p b o s -> p (b o s)')
            gtot = smalls.tile([128, 12], F32, tag='gtot')
            nc.vector.tensor_tensor(out=gtot, in0=gAf, in1=zg,
                                    op=mybir.AluOpType.add)
            nc.vector.tensor_tensor(out=gtot, in0=gtot, in1=gBf,
                                    op=mybir.AluOpType.add)
            g3 = gtot.rearrange('p (b o s) -> p b o s', o=2, s=2)
            gstat = smalls.tile([128, 3, 2], F32, tag='gstat')
            nc.vector.tensor_tensor(out=gstat, in0=g3[:, :, 0], in1=g3[:, :, 1],
                                    op=mybir.AluOpType.add)
            mv = smalls.tile([128, 3, 2], F32, tag='mv')
            nc.vector.tensor_scalar_mul(out=mv, in0=gstat,
                                        scalar1=1.0 / M_TOTAL)
            var = smalls.tile([128, 3], F32, tag='var')
            nc.vector.tensor_tensor(out=var, in0=mv[:, :, 0], in1=mv[:, :, 0],
                                    op=mybir.AluOpType.mult)
            nc.vector.tensor_tensor(out=var, in0=mv[:, :, 1], in1=var,
                                    op=mybir.AluOpType.subtract)
            sd = smalls.tile([128, 3], F32, tag='sd')
            nc.scalar.activation(out=sd, in_=var,
                                 func=mybir.ActivationFunctionType.Sqrt,
                                 bias=epst)
            rstd = smalls.tile([128, 3], F32, tag='rstd')
            nc.vector.reciprocal(out=rstd, in_=sd)
            sc3 = smalls.tile([128, 3], F32, tag='sc3')
            nc.vector.tensor_tensor(out=sc3, in0=gb_sb[:, 0, :], in1=rstd,
                                    op=mybir.AluOpType.mult)
            tmpb = smalls.tile([128, 3], F32, tag='tmpb')
            nc.vector.tensor_tensor(out=tmpb, in0=mv[:, :, 0], in1=sc3,
                                    op=mybir.AluOpType.mult)
            bi3 = smalls.tile([128, 3], F32, tag='bi3')
            nc.vector.tensor_tensor(out=bi3, in0=gb_sb[:, 1, :], in1=tmpb,
                                    op=mybir.AluOpType.subtract)
            # normalize (DVE bf16 4x) + contiguous bf16 stores
            for bi in range(3):
                for p in range(2):
                    oi = 2 * bi + p
                    nrm = nrmp.tile([128, HW], BF16, tag='nrm',
                                    name=f'nrm_{oi}')
                    nc.vector.tensor_scalar(
                        out=nrm, in0=otiles[oi],
                        scalar1=sc3[:, bi:bi + 1], scalar2=bi3[:, bi:bi + 1],
                        op0=mybir.AluOpType.mult, op1=mybir.AluOpType.add)
                    nc.sync.dma_start(out=out[oi], in_=nrm)

        fold_att(0)
        aggs(0)
        conv_pair(0, 0)
        fold_att(1)
        conv_pair(0, 1)
        aggs(1)
        conv_pair(1, 0)
        fold_att(2)
        conv_pair(1, 1)
        aggs(2)
        conv_pair(2, 0)
        conv_pair(2, 1)
        collective_all()
        consume_all(sqs[5])


_NC_CACHE = None


def _get_nc():
    global _NC_CACHE
    if _NC_CACHE is None:
        _NC_CACHE = _build_nc()
    return _NC_CACHE


def _host_weights(w, taps):
    """w [K, O, Cin, kh, kw] -> [128, K, ntap*64] bf16, halves duplicated."""
    k, o, cin, kh, kw = w.shape
    ntap = len(taps)
    wt = np.zeros((k, 128, ntap * C), np.float32)
    for j, (dy, dx) in enumerate(taps):
        blk = w[:, :, :, dy, dx].transpose(0, 2, 1)   # [K, Cin, O]
        wt[:, 0:64, j * C:(j + 1) * C] = blk
        wt[:, 64:128, j * C:(j + 1) * C] = blk
    return np.ascontiguousarray(
        wt.transpose(1, 0, 2)).astype(ml_dtypes.bfloat16)


def _prep_in_maps(inputs):
    x = np.ascontiguousarray(inputs['x'], dtype=np.float32)
    n_total = x.shape[0]
    pads = [(1, 1), (1, 0), (0, 1)]
    xpad = []
    for bi, (bn, (ph, pw), taps) in enumerate(BR):
        ph_, pw_ = pads[bi]
        sl = x[:, C * (bi + 1):C * (bi + 2)]
        p = np.zeros((n_total, C, ph * pw + ROW_SLACK), ml_dtypes.bfloat16)
        img = p[:, :, :ph * pw].reshape(n_total, C, ph, pw)
        img[:, :, ph_:ph_ + H, pw_:pw_ + W] = sl.astype(ml_dtypes.bfloat16)
        xpad.append(np.ascontiguousarray(p))

    shared = {}
    names = [('sq', 'w_sq', 'att_w_sq', 'att_b_sq', 'g_sq', 'b_sq'),
             ('v', 'w_v', 'att_w_v', 'att_b_v', 'g_v', 'b_v'),
             ('h', 'w_h', 'att_w_h', 'att_b_h', 'g_h', 'b_h')]
    att_fold = np.zeros((128, 3, M_FOLD), np.float32)
    att_bias = np.zeros((M_FOLD, 3), np.float32)
    gb_all = np.zeros((128, 2, 3), np.float32)
    for bi, (bn, wk, awk, abk, gk, bk) in enumerate(names):
        w = np.asarray(inputs[wk], dtype=np.float32)
        shared[f'w_{bn}'] = _host_weights(w, BR[bi][2])
        aw = np.asarray(inputs[awk], np.float32)          # [K, C]
        att_fold[0:64, bi, 0:KEXP] = aw.T / float(HW)
        att_fold[64:128, bi, KEXP:2 * KEXP] = aw.T / float(HW)
        ab = np.asarray(inputs[abk], np.float32)          # [K]
        att_bias[0:KEXP, bi] = ab
        att_bias[KEXP:2 * KEXP, bi] = ab
        g_ = np.asarray(inputs[gk], np.float32)
        b_ = np.asarray(inputs[bk], np.float32)
        gb_all[0:64, 0, bi] = g_
        gb_all[64:128, 0, bi] = g_
        gb_all[0:64, 1, bi] = b_
        gb_all[64:128, 1, bi] = b_
    shared['att_fold'] = att_fold.astype(ml_dtypes.bfloat16)
    shared['att_bias'] = att_bias
    shared['gb'] = gb_all

    in_maps = []
    for ci in range(N_CORES):
        m = dict(shared)
        sl = slice(ci * NS, (ci + 1) * NS)
        for bi, (bn, _, _) in enumerate(BR):
            m[f'xp_{bn}'] = xpad[bi][sl]
        in_maps.append(m)
    return in_maps


# out channel oc <- concat channel (oc % 8) * 32 + oc // 8  (shuffle, g=8)
_SHUF_SRC = (np.arange(256) % 8) * 32 + np.arange(256) // 8


def _assemble(inputs, core_outs):
    """core_outs[ci]: [6, 128, HW] bf16 -> full [32, 256, 56, 56] f32."""
    x = np.asarray(inputs['x'], dtype=np.float32)
    n_total = x.shape[0]
    concat = np.empty((n_total, 256, HW), np.float32)
    concat[:, 0:C] = x[:, 0:C].reshape(n_total, C, HW)
    for ci in range(N_CORES):
        ob = np.asarray(core_outs[ci], dtype=np.float32)  # [6, 128, HW]
        for bi in range(3):
            for p in range(2):
                o = ob[2 * bi + p]
                s0 = ci * NS + 2 * p
                concat[s0, C * (bi + 1):C * (bi + 2)] = o[0:64]
                concat[s0 + 1, C * (bi + 1):C * (bi + 2)] = o[64:128]
    full = concat[:, _SHUF_SRC].reshape(n_total, 256, H, W)
    return np.ascontiguousarray(full)


def run_raw(inputs, trace=False, **kwargs):
    """Build+run; returns (full_output, BassKernelResults)."""
    nc = _get_nc()
    in_maps = _prep_in_maps(inputs)
    res = bass_utils.run_bass_kernel_spmd(
        nc, in_maps, core_ids=list(range(N_CORES)), trace=trace, **kwargs)
    full = _assemble(inputs, [res.results[i]['out'] for i in range(N_CORES)])
    return full, res


def kernel(**inputs):
    full, _ = run_raw(inputs)
    return full


# revision 33
# speedup vs baseline: 1.1602x; 1.1602x over previous
"""Trainium2 Bass kernel for nn_BasicNet (CondConv 3-branch + BN + channel shuffle).

v17 design (v3 baseline 303-334us -> ~150us measured, +-15us launch-skew
run variance):
  - sample-pair packing: each input tile holds two samples on the partition
    halves; per-tap matmuls use block-diagonal aggregated weights (K=128,
    M=128) so both samples' outputs fill one PSUM bank per tile. Plain
    matmuls only (tile_position col-tiling crashes this stack), one
    accumulation group per bank. Halves the input DMA vs shifted-copy
    tap pairing and gives full 128-partition evac/stats/normalize ops.
  - attention pooling on the PE: blockdiag(att_w/HW) as a [128, 32]
    stationary operand accumulated over free-dim chunks in PSUM, then
    sigmoid and a DVE 32x32 transpose + partition_broadcast (no per-unit
    gather DMAs). Folds are emitted between conv blocks so each branch's
    attention/aggregation hides under the previous branch's convs.
  - PSUM evacuation at 128 partitions with accum_out channel sums on ACT;
    sum-of-squares via DVE scalar_tensor_tensor+accum_out on the stored
    bf16 (tensor_tensor_reduce crashes on HW).
  - one BN-stats AllReduce (6KB) after all staging, plus an early 16B
    dummy collective that soaks up the cc-ring warmup (~50us) and most of
    the cross-core launch skew while the convs run. The post-collective
    phase is data-gated on the last sumsq so the list scheduler (whose
    cost model thinks collectives are fast) cannot hoist it in front of
    producer work and stall engine queues behind the collective.
  - consume phase vectorized across all 3 branches: two [128,3,2,2] loads
    (straight + partition-halves swapped so the fold needs no
    cross-partition hops), one fused scale/bias chain, 6 normalizes
    (DVE bf16 4x tensor_scalar) and 6 contiguous bf16 stores.
  - bf16 off-PE everywhere: weights + agg tree (DVE 2x/4x modes),
    normalize, stores; host does upcast + channel shuffle + s0 passthrough
    (layout only).
"""

import sys

if '/opt/trn_rl_repo' not in sys.path:
    sys.path.insert(0, '/opt/trn_rl_repo')

import numpy as np
import ml_dtypes

import concourse.bass as bass
import concourse.bacc as bacc
import concourse.tile as tile
from concourse import mybir
from concourse import bass_utils

F32 = mybir.dt.float32
BF16 = mybir.dt.bfloat16

N_CORES = 8
NS = 4                   # samples per core
H = W = 56
HW = H * W               # 3136
C = 64                   # channels per branch (Cin == O == 64)
KEXP = 4                 # CondConv experts
RPT = 8                  # rows per conv tile
NT = RPT * W             # 448 free elements per matmul tile
N_TILES = H // RPT       # 7
M_TOTAL = 32 * HW        # BN stat count (global batch)
EPS = 1e-5
ROW_SLACK = 64           # zero tail per channel row
FOLD_CHUNK = 512
M_FOLD = 32              # fold matmul M (2 samples x 4 experts, zero-padded)

# branch geometry: (name, padded (ph,pw), taps)
BR = [
    ('sq', (58, 58), [(dy, dx) for dy in range(3) for dx in range(3)]),
    ('v', (58, 56), [(0, 0), (1, 0), (2, 0)]),
    ('h', (56, 58), [(0, 0), (0, 1), (0, 2)]),
]


def _build_nc():
    nc = bacc.Bacc('TRN2', target_bir_lowering=False, debug=False,
                   num_devices=N_CORES)

    xp = {}
    w_t = {}
    for bi, (bn, (ph, pw), taps) in enumerate(BR):
        xp[bi] = nc.dram_tensor(f'xp_{bn}', [NS, C, ph * pw + ROW_SLACK], BF16,
                                kind='ExternalInput').ap()
        w_t[bi] = nc.dram_tensor(f'w_{bn}', [128, KEXP, len(taps) * C], BF16,
                                 kind='ExternalInput').ap()
    att_fold = nc.dram_tensor('att_fold', [128, 3, M_FOLD], BF16,
                              kind='ExternalInput').ap()
    att_bias = nc.dram_tensor('att_bias', [M_FOLD, 3], F32,
                              kind='ExternalInput').ap()
    gb = nc.dram_tensor('gb', [128, 2, 3], F32, kind='ExternalInput').ap()
    out = nc.dram_tensor('out', [6, 128, HW], BF16,
                         kind='ExternalOutput').ap()

    with tile.TileContext(nc) as tc:
        _emit(tc, xp, w_t, att_fold, att_bias, gb, out)

    nc.compile()
    return nc


def _emit(tc, xp, w_t, att_fold, att_bias, gb, out):
    nc = tc.nc
    from contextlib import ExitStack
    ctx = ExitStack()
    with ctx:
        persist = ctx.enter_context(tc.tile_pool(name='persist', bufs=1))
        smalls = ctx.enter_context(tc.tile_pool(name='smalls', bufs=4))
        nrmp = ctx.enter_context(tc.tile_pool(name='nrmp', bufs=2))
        scrp = ctx.enter_context(tc.tile_pool(name='scrp', bufs=2))
        psum_conv = ctx.enter_context(
            tc.tile_pool(name='psum_conv', bufs=4, space='PSUM'))
        psum_att = ctx.enter_context(
            tc.tile_pool(name='psum_att', bufs=2, space='PSUM'))
        dram = ctx.enter_context(tc.tile_pool(name='dram', bufs=1, space='DRAM'))

        # ---------- ring-warmup dummy collective ----------
        cc_d_in = dram.tile([4, 1], F32, tag='cc_d_in')
        cc_d_out = dram.tile([4, 1], F32, tag='cc_d_out')
        nc.gpsimd.dma_start(out=cc_d_in, in_=att_bias[0:4, 0:1])
        nc.gpsimd.collective_compute(
            'AllReduce', mybir.AluOpType.add,
            replica_groups=[list(range(N_CORES))],
            ins=[cc_d_in.opt()], outs=[cc_d_out.opt()])

        # ---------- constants (SWDGE queue, ahead of the weights) ----------
        att_fold_sb = persist.tile([128, 3, M_FOLD], BF16, tag='att_fold_sb')
        nc.gpsimd.dma_start(out=att_fold_sb, in_=att_fold)
        att_bias_sb = persist.tile([M_FOLD, 3], F32, tag='att_bias_sb')
        nc.gpsimd.dma_start(out=att_bias_sb, in_=att_bias)
        gb_sb = persist.tile([128, 2, 3], F32, tag='gb_sb')
        nc.gpsimd.dma_start(out=gb_sb, in_=gb)
        epst = persist.tile([128, 1], F32, tag='epst')
        nc.vector.memset(epst, EPS)

        # ---------- input tiles: [sample 2p | sample 2p+1] ----------
        in_t = {}
        for bi, (bn, (ph, pw), taps) in enumerate(BR):
            flat = ph * pw
            for p in range(2):
                t = persist.tile([128, flat], BF16, tag=f'in_{bi}_{p}',
                                 name=f'in_{bi}_{p}')
                nc.sync.dma_start(out=t[0:64, :],
                                  in_=xp[bi][2 * p][:, 0:flat])
                nc.sync.dma_start(out=t[64:128, :],
                                  in_=xp[bi][2 * p + 1][:, 0:flat])
                in_t[(bi, p)] = t

        # ---------- weights (SWDGE queue, off the input path) ----------
        w_sb = {}
        for bi, (bn, _, taps) in enumerate(BR):
            t = persist.tile([128, KEXP, len(taps) * C], BF16,
                             tag=f'w_sb_{bi}', name=f'w_sb_{bi}')
            nc.gpsimd.dma_start(out=t, in_=w_t[bi])
            w_sb[bi] = t

        # block-diagonal aggregated weights, one per (branch, pair); the
        # off-diagonal halves are zeroed once and never rewritten
        ag_t = {}
        for bi, (bn, _, taps) in enumerate(BR):
            for p in range(2):
                t = persist.tile([128, len(taps) * 2 * C], BF16,
                                 tag=f'ag_{bi}_{p}', name=f'ag_{bi}_{p}')
                if bi == 0:
                    nc.gpsimd.memset(t, 0.0)
                ag_t[(bi, p)] = t

        # ---------- persistent outputs / stats ----------
        otiles = [persist.tile([128, HW], BF16, tag=f'ot_{i}', name=f'ot_{i}')
                  for i in range(6)]
        sums = [persist.tile([128, N_TILES], F32, tag=f'sums_{i}',
                             name=f'sums_{i}') for i in range(6)]
        sqs = [persist.tile([128, N_TILES], F32, tag=f'sqs_{i}',
                            name=f'sqs_{i}') for i in range(6)]

        cc_in_all = dram.tile([3, 2, 128, 2], F32, tag='cc_in_all')
        cc_out_all = dram.tile([3, 2, 128, 2], F32, tag='cc_out_all')
        cc_in = [cc_in_all[b] for b in range(3)]
        cc_out = [cc_out_all[b] for b in range(3)]

        # ---------- branches ----------
        # fold_att(b): PE pooled fold + sigmoid + DVE-transpose broadcast.
        # aggs(b): block-diagonal aggregated weights on DVE.
        # conv_pair(b, p): convs + evac + stats + accumulating cc staging.
        # consume(b): collective result -> scale/bias -> normalize -> store.
        # Emission interleaves these so folds hide between conv blocks and no
        # engine's program stalls on a collective while conv work remains.
        att_bcs = {}

        def fold_att(bi):
            bn, (ph, pw), taps = BR[bi]
            flat = ph * pw
            nch = (flat + FOLD_CHUNK - 1) // FOLD_CHUNK
            if bi > 0:
                for p in range(2):
                    nc.gpsimd.memset(ag_t[(bi, p)], 0.0)
            for p in range(2):
                att_ps = psum_att.tile([M_FOLD, FOLD_CHUNK], F32, tag='att_ps')
                for ci in range(nch):
                    c0 = ci * FOLD_CHUNK
                    c1 = min(flat, c0 + FOLD_CHUNK)
                    nc.tensor.matmul(
                        att_ps[:, 0:c1 - c0],
                        lhsT=att_fold_sb[:, bi, :],
                        rhs=in_t[(bi, p)][:, c0:c1],
                        start=(ci == 0), stop=(ci == nch - 1))
                att32 = smalls.tile([M_FOLD, M_FOLD], F32, tag='att32',
                                    name=f'att32_{bi}_{p}')
                nc.vector.memset(att32, 0.0)
                attv = smalls.tile([M_FOLD, 1], F32, tag='attv')
                nc.vector.tensor_reduce(out=attv, in_=att_ps,
                                        axis=mybir.AxisListType.X,
                                        op=mybir.AluOpType.add)
                nc.scalar.activation(out=att32[:, 0:1], in_=attv,
                                     func=mybir.ActivationFunctionType.Sigmoid,
                                     bias=att_bias_sb[:, bi:bi + 1])
                att_t = smalls.tile([M_FOLD, M_FOLD], F32, tag='att_t',
                                    name=f'att_t_{bi}_{p}')
                nc.vector.transpose(out=att_t, in_=att32)
                att_bc = smalls.tile([128, 2 * KEXP], F32, tag='att_bc',
                                     name=f'att_bc_{bi}_{p}')
                nc.gpsimd.partition_broadcast(att_bc, att_t[0:1, 0:2 * KEXP])
                att_bcs[(bi, p)] = att_bc

        def aggs(bi):
            # att_bc col for (half h, expert k) = 4h + k
            for p in range(2):
                ag = ag_t[(bi, p)]
                att_bc = att_bcs[(bi, p)]
                ag4 = ag.rearrange('p (t two o) -> p t two o', two=2, o=C)
                w4 = w_sb[bi].rearrange('p k (t o) -> p k t o', o=C)
                for h in range(2):
                    dst = ag4[64 * h:64 * h + 64, :, h, :]
                    for k in range(KEXP):
                        col = 4 * h + k
                        src = w4[64 * h:64 * h + 64, k]
                        if k == 0:
                            nc.vector.tensor_scalar_mul(
                                out=dst, in0=src,
                                scalar1=att_bc[64 * h:64 * h + 64,
                                               col:col + 1])
                        else:
                            nc.vector.scalar_tensor_tensor(
                                out=dst, in0=src,
                                scalar=att_bc[64 * h:64 * h + 64,
                                              col:col + 1],
                                in1=dst, op0=mybir.AluOpType.mult,
                                op1=mybir.AluOpType.add)

        def conv_pair(bi, p):
            bn, (ph, pw), taps = BR[bi]
            ntap = len(taps)
            oi = 2 * bi + p
            otile = otiles[oi]
            it3 = in_t[(bi, p)].rearrange('c (r q) -> c r q', q=pw)
            ag = ag_t[(bi, p)]
            for t in range(N_TILES):
                pt = psum_conv.tile([128, NT], F32, tag='pt')
                for ti, (dy, dx) in enumerate(taps):
                    r0 = RPT * t + dy
                    nc.tensor.matmul(
                        pt, lhsT=ag[:, ti * 2 * C:(ti + 1) * 2 * C],
                        rhs=it3[:, r0:r0 + RPT, dx:dx + W],
                        start=(ti == 0), stop=(ti == ntap - 1))
                # evacuate (bf16) + channel sums on ACT
                nc.scalar.activation(
                    out=otile[:, t * NT:(t + 1) * NT], in_=pt,
                    func=mybir.ActivationFunctionType.Copy,
                    accum_out=sums[oi][:, t:t + 1])
                # sum of squares on DVE from the stored bf16
                scr = scrp.tile([128, NT], BF16, tag='scr')
                osl = otile[:, t * NT:(t + 1) * NT]
                nc.vector.scalar_tensor_tensor(
                    out=scr, in0=osl, scalar=1.0, in1=osl,
                    op0=mybir.AluOpType.mult, op1=mybir.AluOpType.mult,
                    accum_out=sqs[oi][:, t:t + 1])
            # per-otile totals; the four partition-half blocks accumulate
            # into one [64, 2] cc buffer via DMA-compute (first overwrites)
            red = smalls.tile([128, 2], F32, tag='red', name=f'red_{oi}')
            nc.vector.tensor_reduce(out=red[:, 0:1], in_=sums[oi],
                                    axis=mybir.AxisListType.X,
                                    op=mybir.AluOpType.add)
            nc.vector.tensor_reduce(out=red[:, 1:2], in_=sqs[oi],
                                    axis=mybir.AxisListType.X,
                                    op=mybir.AluOpType.add)
            nc.sync.dma_start(out=cc_in[bi][p], in_=red)

        def collective_all():
            nc.gpsimd.collective_compute(
                'AllReduce', mybir.AluOpType.add,
                replica_groups=[list(range(N_CORES))],
                ins=[cc_in_all.opt()], outs=[cc_out_all.opt()])

        def consume_all(gate):
            # One fused post-collective phase for all three branches. The
            # zero-multiplied `gate` (the last sumsq stats) keeps the chain's
            # scheduler readiness behind the producer work so nothing here is
            # hoisted in front of it (the list scheduler thinks collectives
            # are fast and would otherwise stall engine queues on HW).
            zg = smalls.tile([128, 12], F32, tag='zg')
            nc.vector.tensor_scalar_mul(out=zg[:, 0:6], in0=gate[:, 1:7],
                                        scalar1=0.0)
            nc.vector.tensor_scalar_mul(out=zg[:, 6:12], in0=gate[:, 1:7],
                                        scalar1=0.0)
            # [128, 3(branch), 2(otile), 2(stat)] straight + halves swapped
            cr = cc_out_all.rearrange('b o p s -> p b o s')
            gA = smalls.tile([128, 3, 2, 2], F32, tag='gA')
            nc.sync.dma_start(out=gA, in_=cr)
            gB = smalls.tile([128, 3, 2, 2], F32, tag='gB')
            nc.gpsimd.dma_start(out=gB[0:64], in_=cr[64:128])
            nc.gpsimd.dma_start(out=gB[64:128], in_=cr[0:64])
            gAf = gA.rearrange('p b o s -> p (b o s)')
            gBf = gB.rearrange('p b o s -> p (b o s)')
            gtot = smalls.tile([128, 12], F32, tag='gtot')
            nc.vector.tensor_tensor(out=gtot, in0=gAf, in1=zg,
                                    op=mybir.AluOpType.add)
            nc.vector.tensor_tensor(out=gtot, in0=gtot, in1=gBf,
                                    op=mybir.AluOpType.add)
            g3 = gtot.rearrange('p (b o s) -> p b o s', o=2, s=2)
            gstat = smalls.tile([128, 3, 2], F32, tag='gstat')
            nc.vector.tensor_tensor(out=gstat, in0=g3[:, :, 0], in1=g3[:, :, 1],
                                    op=mybir.AluOpType.add)
            mv = smalls.tile([128, 3, 2], F32, tag='mv')
            nc.vector.tensor_scalar_mul(out=mv, in0=gstat,
                                        scalar1=1.0 / M_TOTAL)
            var = smalls.tile([128, 3], F32, tag='var')
            nc.vector.tensor_tensor(out=var, in0=mv[:, :, 0], in1=mv[:, :, 0],
                                    op=mybir.AluOpType.mult)
            nc.vector.tensor_tensor(out=var, in0=mv[:, :, 1], in1=var,
                                    op=mybir.AluOpType.subtract)
            sd = smalls.tile([128, 3], F32, tag='sd')
            nc.scalar.activation(out=sd, in_=var,
                                 func=mybir.ActivationFunctionType.Sqrt,
                                 bias=epst)
            rstd = smalls.tile([128, 3], F32, tag='rstd')
            nc.vector.reciprocal(out=rstd, in_=sd)
            sc3 = smalls.tile([128, 3], F32, tag='sc3')
            nc.vector.tensor_tensor(out=sc3, in0=gb_sb[:, 0, :], in1=rstd,
                                    op=mybir.AluOpType.mult)
            tmpb = smalls.tile([128, 3], F32, tag='tmpb')
            nc.vector.tensor_tensor(out=tmpb, in0=mv[:, :, 0], in1=sc3,
                                    op=mybir.AluOpType.mult)
            bi3 = smalls.tile([128, 3], F32, tag='bi3')
            nc.vector.tensor_tensor(out=bi3, in0=gb_sb[:, 1, :], in1=tmpb,
                                    op=mybir.AluOpType.subtract)
            # normalize (DVE bf16 4x) + contiguous bf16 stores
            for bi in range(3):
                for p in range(2):
                    oi = 2 * bi + p
                    nrm = nrmp.tile([128, HW], BF16, tag='nrm',
                                    name=f'nrm_{oi}')
                    nc.vector.tensor_scalar(
                        out=nrm, in0=otiles[oi],
                        scalar1=sc3[:, bi:bi + 1], scalar2=bi3[:, bi:bi + 1],
                        op0=mybir.AluOpType.mult, op1=mybir.AluOpType.add)
                    nc.sync.dma_start(out=out[oi], in_=nrm)

        fold_att(0)
        aggs(0)
        conv_pair(0, 0)
        fold_att(1)
        conv_pair(0, 1)
        aggs(1)
        conv_pair(1, 0)
        fold_att(2)
        conv_pair(1, 1)
        aggs(2)
        conv_pair(2, 0)
        conv_pair(2, 1)
        collective_all()
        consume_all(sqs[5])


_NC_CACHE = None


def _get_nc():
    global _NC_CACHE
    if _NC_CACHE is None:
        _NC_CACHE = _build_nc()
    return _NC_CACHE


def _host_weights(w, taps):
    """w [K, O, Cin, kh, kw] -> [128, K, ntap*64] bf16, halves duplicated."""
    k, o, cin, kh, kw = w.shape
    ntap = len(taps)
    wt = np.zeros((k, 128, ntap * C), np.float32)
    for j, (dy, dx) in enumerate(taps):
        blk = w[:, :, :, dy, dx].transpose(0, 2, 1)   # [K, Cin, O]
        wt[:, 0:64, j * C:(j + 1) * C] = blk
        wt[:, 64:128, j * C:(j + 1) * C] = blk
    return np.ascontiguousarray(
        wt.transpose(1, 0, 2)).astype(ml_dtypes.bfloat16)


def _prep_in_maps(inputs):
    x = np.ascontiguousarray(inputs['x'], dtype=np.float32)
    n_total = x.shape[0]
    pads = [(1, 1), (1, 0), (0, 1)]
    xpad = []
    for bi, (bn, (ph, pw), taps) in enumerate(BR):
        ph_, pw_ = pads[bi]
        sl = x[:, C * (bi + 1):C * (bi + 2)]
        p = np.zeros((n_total, C, ph * pw + ROW_SLACK), ml_dtypes.bfloat16)
        img = p[:, :, :ph * pw].reshape(n_total, C, ph, pw)
        img[:, :, ph_:ph_ + H, pw_:pw_ + W] = sl.astype(ml_dtypes.bfloat16)
        xpad.append(np.ascontiguousarray(p))

    shared = {}
    names = [('sq', 'w_sq', 'att_w_sq', 'att_b_sq', 'g_sq', 'b_sq'),
             ('v', 'w_v', 'att_w_v', 'att_b_v', 'g_v', 'b_v'),
             ('h', 'w_h', 'att_w_h', 'att_b_h', 'g_h', 'b_h')]
    att_fold = np.zeros((128, 3, M_FOLD), np.float32)
    att_bias = np.zeros((M_FOLD, 3), np.float32)
    gb_all = np.zeros((128, 2, 3), np.float32)
    for bi, (bn, wk, awk, abk, gk, bk) in enumerate(names):
        w = np.asarray(inputs[wk], dtype=np.float32)
        shared[f'w_{bn}'] = _host_weights(w, BR[bi][2])
        aw = np.asarray(inputs[awk], np.float32)          # [K, C]
        att_fold[0:64, bi, 0:KEXP] = aw.T / float(HW)
        att_fold[64:128, bi, KEXP:2 * KEXP] = aw.T / float(HW)
        ab = np.asarray(inputs[abk], np.float32)          # [K]
        att_bias[0:KEXP, bi] = ab
        att_bias[KEXP:2 * KEXP, bi] = ab
        g_ = np.asarray(inputs[gk], np.float32)
        b_ = np.asarray(inputs[bk], np.float32)
        gb_all[0:64, 0, bi] = g_
        gb_all[64:128, 0, bi] = g_
        gb_all[0:64, 1, bi] = b_
        gb_all[64:128, 1, bi] = b_
    shared['att_fold'] = att_fold.astype(ml_dtypes.bfloat16)
    shared['att_bias'] = att_bias
    shared['gb'] = gb_all

    in_maps = []
    for ci in range(N_CORES):
        m = dict(shared)
        sl = slice(ci * NS, (ci + 1) * NS)
        for bi, (bn, _, _) in enumerate(BR):
            m[f'xp_{bn}'] = xpad[bi][sl]
        in_maps.append(m)
    return in_maps


# out channel oc <- concat channel (oc % 8) * 32 + oc // 8  (shuffle, g=8)
_SHUF_SRC = (np.arange(256) % 8) * 32 + np.arange(256) // 8


def _assemble(inputs, core_outs):
    """core_outs[ci]: [6, 128, HW] bf16 -> full [32, 256, 56, 56] f32."""
    x = np.asarray(inputs['x'], dtype=np.float32)
    n_total = x.shape[0]
    concat = np.empty((n_total, 256, HW), np.float32)
    concat[:, 0:C] = x[:, 0:C].reshape(n_total, C, HW)
    for ci in range(N_CORES):
        ob = np.asarray(core_outs[ci], dtype=np.float32)  # [6, 128, HW]
        for bi in range(3):
            for p in range(2):
                o = ob[2 * bi + p]
                s0 = ci * NS + 2 * p
                concat[s0, C * (bi + 1):C * (bi + 2)] = o[0:64]
                concat[s0 + 1, C * (bi + 1):C * (bi + 2)] = o[64:128]
    full = concat[:, _SHUF_SRC].reshape(n_total, 256, H, W)
    return np.ascontiguousarray(full)


def run_raw(inputs, trace=False, **kwargs):
    """Build+run; returns (full_output, BassKernelResults)."""
    nc = _get_nc()
    in_maps = _prep_in_maps(inputs)
    res = bass_utils.run_bass_kernel_spmd(
        nc, in_maps, core_ids=list(range(N_CORES)), trace=trace, **kwargs)
    full = _assemble(inputs, [res.results[i]['out'] for i in range(N_CORES)])
    return full, res


def kernel(**inputs):
    full, _ = run_raw(inputs)
    return full


# revision 34
# speedup vs baseline: 1.1999x; 1.0343x over previous
"""Trainium2 Bass kernel for nn_BasicNet (CondConv 3-branch + BN + channel shuffle).

v17 design (v3 baseline 303-334us -> ~150us measured, +-15us launch-skew
run variance):
  - sample-pair packing: each input tile holds two samples on the partition
    halves; per-tap matmuls use block-diagonal aggregated weights (K=128,
    M=128) so both samples' outputs fill one PSUM bank per tile. Plain
    matmuls only (tile_position col-tiling crashes this stack), one
    accumulation group per bank. Halves the input DMA vs shifted-copy
    tap pairing and gives full 128-partition evac/stats/normalize ops.
  - attention pooling on the PE: blockdiag(att_w/HW) as a [128, 32]
    stationary operand accumulated over free-dim chunks in PSUM, then
    sigmoid and a DVE 32x32 transpose + partition_broadcast (no per-unit
    gather DMAs). Folds are emitted between conv blocks so each branch's
    attention/aggregation hides under the previous branch's convs.
  - PSUM evacuation at 128 partitions with accum_out channel sums on ACT;
    sum-of-squares via DVE scalar_tensor_tensor+accum_out on the stored
    bf16 (tensor_tensor_reduce crashes on HW).
  - one BN-stats AllReduce (6KB) after all staging, plus an early 16B
    dummy collective that soaks up the cc-ring warmup (~50us) and most of
    the cross-core launch skew while the convs run. The post-collective
    phase is data-gated on the last sumsq so the list scheduler (whose
    cost model thinks collectives are fast) cannot hoist it in front of
    producer work and stall engine queues behind the collective.
  - consume phase vectorized across all 3 branches: two [128,3,2,2] loads
    (straight + partition-halves swapped so the fold needs no
    cross-partition hops), one fused scale/bias chain, 6 normalizes
    (DVE bf16 4x tensor_scalar) and 6 contiguous bf16 stores.
  - bf16 off-PE everywhere: weights + agg tree (DVE 2x/4x modes),
    normalize, stores; host does upcast + channel shuffle + s0 passthrough
    (layout only).
"""

import sys

if '/opt/trn_rl_repo' not in sys.path:
    sys.path.insert(0, '/opt/trn_rl_repo')

import numpy as np
import ml_dtypes

import concourse.bass as bass
import concourse.bacc as bacc
import concourse.tile as tile
from concourse import mybir
from concourse import bass_utils

F32 = mybir.dt.float32
BF16 = mybir.dt.bfloat16

N_CORES = 8
NS = 4                   # samples per core
H = W = 56
HW = H * W               # 3136
C = 64                   # channels per branch (Cin == O == 64)
KEXP = 4                 # CondConv experts
RPT = 8                  # rows per conv tile
NT = RPT * W             # 448 free elements per matmul tile
N_TILES = H // RPT       # 7
M_TOTAL = 32 * HW        # BN stat count (global batch)
EPS = 1e-5
ROW_SLACK = 64           # zero tail per channel row
FOLD_CHUNK = 512
M_FOLD = 32              # fold matmul M (2 samples x 4 experts, zero-padded)

# branch geometry: (name, padded (ph,pw), taps)
BR = [
    ('sq', (58, 58), [(dy, dx) for dy in range(3) for dx in range(3)]),
    ('v', (58, 56), [(0, 0), (1, 0), (2, 0)]),
    ('h', (56, 58), [(0, 0), (0, 1), (0, 2)]),
]


def _build_nc():
    nc = bacc.Bacc('TRN2', target_bir_lowering=False, debug=False,
                   num_devices=N_CORES)

    xp = {}
    w_t = {}
    for bi, (bn, (ph, pw), taps) in enumerate(BR):
        xp[bi] = nc.dram_tensor(f'xp_{bn}', [NS, C, ph * pw + ROW_SLACK], BF16,
                                kind='ExternalInput').ap()
        w_t[bi] = nc.dram_tensor(f'w_{bn}', [128, KEXP, len(taps) * C], BF16,
                                 kind='ExternalInput').ap()
    att_fold = nc.dram_tensor('att_fold', [128, 3, M_FOLD], BF16,
                              kind='ExternalInput').ap()
    att_bias = nc.dram_tensor('att_bias', [M_FOLD, 3], F32,
                              kind='ExternalInput').ap()
    gb = nc.dram_tensor('gb', [128, 2, 3], F32, kind='ExternalInput').ap()
    out = nc.dram_tensor('out', [6, 128, HW], BF16,
                         kind='ExternalOutput').ap()

    with tile.TileContext(nc) as tc:
        _emit(tc, xp, w_t, att_fold, att_bias, gb, out)

    nc.compile()
    return nc


def _emit(tc, xp, w_t, att_fold, att_bias, gb, out):
    nc = tc.nc
    from contextlib import ExitStack
    ctx = ExitStack()
    with ctx:
        persist = ctx.enter_context(tc.tile_pool(name='persist', bufs=1))
        smalls = ctx.enter_context(tc.tile_pool(name='smalls', bufs=4))
        nrmp = ctx.enter_context(tc.tile_pool(name='nrmp', bufs=2))
        scrp = ctx.enter_context(tc.tile_pool(name='scrp', bufs=2))
        psum_conv = ctx.enter_context(
            tc.tile_pool(name='psum_conv', bufs=4, space='PSUM'))
        psum_att = ctx.enter_context(
            tc.tile_pool(name='psum_att', bufs=2, space='PSUM'))
        dram = ctx.enter_context(tc.tile_pool(name='dram', bufs=1, space='DRAM'))

        # ---------- ring-warmup dummy collective ----------
        cc_d_in = dram.tile([4, 1], F32, tag='cc_d_in')
        cc_d_out = dram.tile([4, 1], F32, tag='cc_d_out')
        nc.gpsimd.dma_start(out=cc_d_in, in_=att_bias[0:4, 0:1])
        nc.gpsimd.collective_compute(
            'AllReduce', mybir.AluOpType.add,
            replica_groups=[list(range(N_CORES))],
            ins=[cc_d_in.opt()], outs=[cc_d_out.opt()])

        # ---------- constants (SWDGE queue, ahead of the weights) ----------
        att_fold_sb = persist.tile([128, 3, M_FOLD], BF16, tag='att_fold_sb')
        nc.gpsimd.dma_start(out=att_fold_sb, in_=att_fold)
        att_bias_sb = persist.tile([M_FOLD, 3], F32, tag='att_bias_sb')
        nc.gpsimd.dma_start(out=att_bias_sb, in_=att_bias)
        gb_sb = persist.tile([128, 2, 3], F32, tag='gb_sb')
        nc.gpsimd.dma_start(out=gb_sb, in_=gb)
        epst = persist.tile([128, 1], F32, tag='epst')
        nc.vector.memset(epst, EPS)

        # ---------- input tiles: [sample 2p | sample 2p+1] ----------
        in_t = {}
        for bi, (bn, (ph, pw), taps) in enumerate(BR):
            flat = ph * pw
            for p in range(2):
                t = persist.tile([128, flat], BF16, tag=f'in_{bi}_{p}',
                                 name=f'in_{bi}_{p}')
                nc.sync.dma_start(out=t[0:64, :],
                                  in_=xp[bi][2 * p][:, 0:flat])
                nc.sync.dma_start(out=t[64:128, :],
                                  in_=xp[bi][2 * p + 1][:, 0:flat])
                in_t[(bi, p)] = t

        # ---------- weights (SWDGE queue, off the input path) ----------
        w_sb = {}
        for bi, (bn, _, taps) in enumerate(BR):
            t = persist.tile([128, KEXP, len(taps) * C], BF16,
                             tag=f'w_sb_{bi}', name=f'w_sb_{bi}')
            nc.gpsimd.dma_start(out=t, in_=w_t[bi])
            w_sb[bi] = t

        # block-diagonal aggregated weights, one per (branch, pair); the
        # off-diagonal halves are zeroed once and never rewritten
        ag_t = {}
        for bi, (bn, _, taps) in enumerate(BR):
            for p in range(2):
                t = persist.tile([128, len(taps) * 2 * C], BF16,
                                 tag=f'ag_{bi}_{p}', name=f'ag_{bi}_{p}')
                if bi == 0:
                    nc.gpsimd.memset(t, 0.0)
                ag_t[(bi, p)] = t

        # ---------- persistent outputs / stats ----------
        otiles = [persist.tile([128, HW], BF16, tag=f'ot_{i}', name=f'ot_{i}')
                  for i in range(6)]
        sums = [persist.tile([128, N_TILES], F32, tag=f'sums_{i}',
                             name=f'sums_{i}') for i in range(6)]
        sqs = [persist.tile([128, N_TILES], F32, tag=f'sqs_{i}',
                            name=f'sqs_{i}') for i in range(6)]

        cc_in_all = dram.tile([3, 2, 128, 2], F32, tag='cc_in_all')
        cc_out_all = dram.tile([3, 2, 128, 2], F32, tag='cc_out_all')
        cc_in = [cc_in_all[b] for b in range(3)]
        cc_out = [cc_out_all[b] for b in range(3)]

        # ---------- branches ----------
        # fold_att(b): PE pooled fold + sigmoid + DVE-transpose broadcast.
        # aggs(b): block-diagonal aggregated weights on DVE.
        # conv_pair(b, p): convs + evac + stats + accumulating cc staging.
        # consume(b): collective result -> scale/bias -> normalize -> store.
        # Emission interleaves these so folds hide between conv blocks and no
        # engine's program stalls on a collective while conv work remains.
        att_bcs = {}

        def fold_att(bi):
            bn, (ph, pw), taps = BR[bi]
            flat = ph * pw
            nch = (flat + FOLD_CHUNK - 1) // FOLD_CHUNK
            if bi > 0:
                for p in range(2):
                    nc.gpsimd.memset(ag_t[(bi, p)], 0.0)
            for p in range(2):
                att_ps = psum_att.tile([M_FOLD, FOLD_CHUNK], F32, tag='att_ps')
                for ci in range(nch):
                    c0 = ci * FOLD_CHUNK
                    c1 = min(flat, c0 + FOLD_CHUNK)
                    nc.tensor.matmul(
                        att_ps[:, 0:c1 - c0],
                        lhsT=att_fold_sb[:, bi, :],
                        rhs=in_t[(bi, p)][:, c0:c1],
                        start=(ci == 0), stop=(ci == nch - 1))
                att32 = smalls.tile([M_FOLD, M_FOLD], F32, tag='att32',
                                    name=f'att32_{bi}_{p}')
                nc.vector.memset(att32, 0.0)
                attv = smalls.tile([M_FOLD, 1], F32, tag='attv')
                nc.vector.tensor_reduce(out=attv, in_=att_ps,
                                        axis=mybir.AxisListType.X,
                                        op=mybir.AluOpType.add)
                nc.scalar.activation(out=att32[:, 0:1], in_=attv,
                                     func=mybir.ActivationFunctionType.Sigmoid,
                                     bias=att_bias_sb[:, bi:bi + 1])
                att_t = smalls.tile([M_FOLD, M_FOLD], F32, tag='att_t',
                                    name=f'att_t_{bi}_{p}')
                nc.vector.transpose(out=att_t, in_=att32)
                att_bc = smalls.tile([128, 2 * KEXP], F32, tag='att_bc',
                                     name=f'att_bc_{bi}_{p}')
                nc.gpsimd.partition_broadcast(att_bc, att_t[0:1, 0:2 * KEXP])
                att_bcs[(bi, p)] = att_bc

        def aggs(bi):
            # att_bc col for (half h, expert k) = 4h + k
            for p in range(2):
                ag = ag_t[(bi, p)]
                att_bc = att_bcs[(bi, p)]
                ag4 = ag.rearrange('p (t two o) -> p t two o', two=2, o=C)
                w4 = w_sb[bi].rearrange('p k (t o) -> p k t o', o=C)
                for h in range(2):
                    dst = ag4[64 * h:64 * h + 64, :, h, :]
                    for k in range(KEXP):
                        col = 4 * h + k
                        src = w4[64 * h:64 * h + 64, k]
                        if k == 0:
                            nc.vector.tensor_scalar_mul(
                                out=dst, in0=src,
                                scalar1=att_bc[64 * h:64 * h + 64,
                                               col:col + 1])
                        else:
                            nc.vector.scalar_tensor_tensor(
                                out=dst, in0=src,
                                scalar=att_bc[64 * h:64 * h + 64,
                                              col:col + 1],
                                in1=dst, op0=mybir.AluOpType.mult,
                                op1=mybir.AluOpType.add)

        def conv_pair(bi, p):
            bn, (ph, pw), taps = BR[bi]
            ntap = len(taps)
            oi = 2 * bi + p
            otile = otiles[oi]
            it3 = in_t[(bi, p)].rearrange('c (r q) -> c r q', q=pw)
            ag = ag_t[(bi, p)]
            for t in range(N_TILES):
                pt = psum_conv.tile([128, NT], F32, tag='pt')
                for ti, (dy, dx) in enumerate(taps):
                    r0 = RPT * t + dy
                    nc.tensor.matmul(
                        pt, lhsT=ag[:, ti * 2 * C:(ti + 1) * 2 * C],
                        rhs=it3[:, r0:r0 + RPT, dx:dx + W],
                        start=(ti == 0), stop=(ti == ntap - 1))
                # evacuate (bf16) + channel sums on ACT
                nc.scalar.activation(
                    out=otile[:, t * NT:(t + 1) * NT], in_=pt,
                    func=mybir.ActivationFunctionType.Copy,
                    accum_out=sums[oi][:, t:t + 1])
                # sum of squares from the stored bf16, split DVE/ACT by
                # tile parity (both run ~45-75% busy in the conv window)
                scr = scrp.tile([128, NT], BF16, tag='scr')
                osl = otile[:, t * NT:(t + 1) * NT]
                if t % 2 == 0:
                    nc.vector.scalar_tensor_tensor(
                        out=scr, in0=osl, scalar=1.0, in1=osl,
                        op0=mybir.AluOpType.mult, op1=mybir.AluOpType.mult,
                        accum_out=sqs[oi][:, t:t + 1])
                else:
                    nc.scalar.activation(
                        out=scr, in_=osl,
                        func=mybir.ActivationFunctionType.Square,
                        accum_out=sqs[oi][:, t:t + 1])
            # per-otile totals; the four partition-half blocks accumulate
            # into one [64, 2] cc buffer via DMA-compute (first overwrites)
            red = smalls.tile([128, 2], F32, tag='red', name=f'red_{oi}')
            nc.vector.tensor_reduce(out=red[:, 0:1], in_=sums[oi],
                                    axis=mybir.AxisListType.X,
                                    op=mybir.AluOpType.add)
            nc.vector.tensor_reduce(out=red[:, 1:2], in_=sqs[oi],
                                    axis=mybir.AxisListType.X,
                                    op=mybir.AluOpType.add)
            nc.sync.dma_start(out=cc_in[bi][p], in_=red)

        def collective_all():
            nc.gpsimd.collective_compute(
                'AllReduce', mybir.AluOpType.add,
                replica_groups=[list(range(N_CORES))],
                ins=[cc_in_all.opt()], outs=[cc_out_all.opt()])

        def consume_all(gate):
            # One fused post-collective phase for all three branches. The
            # zero-multiplied `gate` (the last sumsq stats) keeps the chain's
            # scheduler readiness behind the producer work so nothing here is
            # hoisted in front of it (the list scheduler thinks collectives
            # are fast and would otherwise stall engine queues on HW).
            zg = smalls.tile([128, 12], F32, tag='zg')
            nc.vector.tensor_scalar_mul(out=zg[:, 0:6], in0=gate[:, 1:7],
                                        scalar1=0.0)
            nc.vector.tensor_scalar_mul(out=zg[:, 6:12], in0=gate[:, 1:7],
                                        scalar1=0.0)
            # [128, 3(branch), 2(otile), 2(stat)] straight + halves swapped
            cr = cc_out_all.rearrange('b o p s -> p b o s')
            gA = smalls.tile([128, 3, 2, 2], F32, tag='gA')
            nc.sync.dma_start(out=gA, in_=cr)
            gB = smalls.tile([128, 3, 2, 2], F32, tag='gB')
            nc.gpsimd.dma_start(out=gB[0:64], in_=cr[64:128])
            nc.gpsimd.dma_start(out=gB[64:128], in_=cr[0:64])
            gAf = gA.rearrange('p b o s -> p (b o s)')
            gBf = gB.rearrange('p b o s -> p (b o s)')
            gtot = smalls.tile([128, 12], F32, tag='gtot')
            nc.vector.tensor_tensor(out=gtot, in0=gAf, in1=zg,
                                    op=mybir.AluOpType.add)
            nc.vector.tensor_tensor(out=gtot, in0=gtot, in1=gBf,
                                    op=mybir.AluOpType.add)
            g3 = gtot.rearrange('p (b o s) -> p b o s', o=2, s=2)
            gstat = smalls.tile([128, 3, 2], F32, tag='gstat')
            nc.vector.tensor_tensor(out=gstat, in0=g3[:, :, 0], in1=g3[:, :, 1],
                                    op=mybir.AluOpType.add)
            mv = smalls.tile([128, 3, 2], F32, tag='mv')
            nc.vector.tensor_scalar_mul(out=mv, in0=gstat,
                                        scalar1=1.0 / M_TOTAL)
            var = smalls.tile([128, 3], F32, tag='var')
            nc.vector.tensor_tensor(out=var, in0=mv[:, :, 0], in1=mv[:, :, 0],
                                    op=mybir.AluOpType.mult)
            nc.vector.tensor_tensor(out=var, in0=mv[:, :, 1], in1=var,
                                    op=mybir.AluOpType.subtract)
            sd = smalls.tile([128, 3], F32, tag='sd')
            nc.scalar.activation(out=sd, in_=var,
                                 func=mybir.ActivationFunctionType.Sqrt,
                                 bias=epst)
            rstd = smalls.tile([128, 3], F32, tag='rstd')
            nc.vector.reciprocal(out=rstd, in_=sd)
            sc3 = smalls.tile([128, 3], F32, tag='sc3')
            nc.vector.tensor_tensor(out=sc3, in0=gb_sb[:, 0, :], in1=rstd,
                                    op=mybir.AluOpType.mult)
            tmpb = smalls.tile([128, 3], F32, tag='tmpb')
            nc.vector.tensor_tensor(out=tmpb, in0=mv[:, :, 0], in1=sc3,
                                    op=mybir.AluOpType.mult)
            bi3 = smalls.tile([128, 3], F32, tag='bi3')
            nc.vector.tensor_tensor(out=bi3, in0=gb_sb[:, 1, :], in1=tmpb,
                                    op=mybir.AluOpType.subtract)
            # normalize (DVE bf16 4x) + contiguous bf16 stores
            for bi in range(3):
                for p in range(2):
                    oi = 2 * bi + p
                    nrm = nrmp.tile([128, HW], BF16, tag='nrm',
                                    name=f'nrm_{oi}')
                    nc.vector.tensor_scalar(
                        out=nrm, in0=otiles[oi],
                        scalar1=sc3[:, bi:bi + 1], scalar2=bi3[:, bi:bi + 1],
                        op0=mybir.AluOpType.mult, op1=mybir.AluOpType.add)
                    nc.sync.dma_start(out=out[oi], in_=nrm)

        fold_att(0)
        aggs(0)
        conv_pair(0, 0)
        fold_att(1)
        conv_pair(0, 1)
        aggs(1)
        conv_pair(1, 0)
        fold_att(2)
        conv_pair(1, 1)
        aggs(2)
        conv_pair(2, 0)
        conv_pair(2, 1)
        collective_all()
        consume_all(sqs[5])


_NC_CACHE = None


def _get_nc():
    global _NC_CACHE
    if _NC_CACHE is None:
        _NC_CACHE = _build_nc()
    return _NC_CACHE


def _host_weights(w, taps):
    """w [K, O, Cin, kh, kw] -> [128, K, ntap*64] bf16, halves duplicated."""
    k, o, cin, kh, kw = w.shape
    ntap = len(taps)
    wt = np.zeros((k, 128, ntap * C), np.float32)
    for j, (dy, dx) in enumerate(taps):
        blk = w[:, :, :, dy, dx].transpose(0, 2, 1)   # [K, Cin, O]
        wt[:, 0:64, j * C:(j + 1) * C] = blk
        wt[:, 64:128, j * C:(j + 1) * C] = blk
    return np.ascontiguousarray(
        wt.transpose(1, 0, 2)).astype(ml_dtypes.bfloat16)


def _prep_in_maps(inputs):
    x = np.ascontiguousarray(inputs['x'], dtype=np.float32)
    n_total = x.shape[0]
    pads = [(1, 1), (1, 0), (0, 1)]
    xpad = []
    for bi, (bn, (ph, pw), taps) in enumerate(BR):
        ph_, pw_ = pads[bi]
        sl = x[:, C * (bi + 1):C * (bi + 2)]
        p = np.zeros((n_total, C, ph * pw + ROW_SLACK), ml_dtypes.bfloat16)
        img = p[:, :, :ph * pw].reshape(n_total, C, ph, pw)
        img[:, :, ph_:ph_ + H, pw_:pw_ + W] = sl.astype(ml_dtypes.bfloat16)
        xpad.append(np.ascontiguousarray(p))

    shared = {}
    names = [('sq', 'w_sq', 'att_w_sq', 'att_b_sq', 'g_sq', 'b_sq'),
             ('v', 'w_v', 'att_w_v', 'att_b_v', 'g_v', 'b_v'),
             ('h', 'w_h', 'att_w_h', 'att_b_h', 'g_h', 'b_h')]
    att_fold = np.zeros((128, 3, M_FOLD), np.float32)
    att_bias = np.zeros((M_FOLD, 3), np.float32)
    gb_all = np.zeros((128, 2, 3), np.float32)
    for bi, (bn, wk, awk, abk, gk, bk) in enumerate(names):
        w = np.asarray(inputs[wk], dtype=np.float32)
        shared[f'w_{bn}'] = _host_weights(w, BR[bi][2])
        aw = np.asarray(inputs[awk], np.float32)          # [K, C]
        att_fold[0:64, bi, 0:KEXP] = aw.T / float(HW)
        att_fold[64:128, bi, KEXP:2 * KEXP] = aw.T / float(HW)
        ab = np.asarray(inputs[abk], np.float32)          # [K]
        att_bias[0:KEXP, bi] = ab
        att_bias[KEXP:2 * KEXP, bi] = ab
        g_ = np.asarray(inputs[gk], np.float32)
        b_ = np.asarray(inputs[bk], np.float32)
        gb_all[0:64, 0, bi] = g_
        gb_all[64:128, 0, bi] = g_
        gb_all[0:64, 1, bi] = b_
        gb_all[64:128, 1, bi] = b_
    shared['att_fold'] = att_fold.astype(ml_dtypes.bfloat16)
    shared['att_bias'] = att_bias
    shared['gb'] = gb_all

    in_maps = []
    for ci in range(N_CORES):
        m = dict(shared)
        sl = slice(ci * NS, (ci + 1) * NS)
        for bi, (bn, _, _) in enumerate(BR):
            m[f'xp_{bn}'] = xpad[bi][sl]
        in_maps.append(m)
    return in_maps


# out channel oc <- concat channel (oc % 8) * 32 + oc // 8  (shuffle, g=8)
_SHUF_SRC = (np.arange(256) % 8) * 32 + np.arange(256) // 8


def _assemble(inputs, core_outs):
    """core_outs[ci]: [6, 128, HW] bf16 -> full [32, 256, 56, 56] f32."""
    x = np.asarray(inputs['x'], dtype=np.float32)
    n_total = x.shape[0]
    concat = np.empty((n_total, 256, HW), np.float32)
    concat[:, 0:C] = x[:, 0:C].reshape(n_total, C, HW)
    for ci in range(N_CORES):
        ob = np.asarray(core_outs[ci], dtype=np.float32)  # [6, 128, HW]
        for bi in range(3):
            for p in range(2):
                o = ob[2 * bi + p]
                s0 = ci * NS + 2 * p
                concat[s0, C * (bi + 1):C * (bi + 2)] = o[0:64]
                concat[s0 + 1, C * (bi + 1):C * (bi + 2)] = o[64:128]
    full = concat[:, _SHUF_SRC].reshape(n_total, 256, H, W)
    return np.ascontiguousarray(full)


def run_raw(inputs, trace=False, **kwargs):
    """Build+run; returns (full_output, BassKernelResults)."""
    nc = _get_nc()
    in_maps = _prep_in_maps(inputs)
    res = bass_utils.run_bass_kernel_spmd(
        nc, in_maps, core_ids=list(range(N_CORES)), trace=trace, **kwargs)
    full = _assemble(inputs, [res.results[i]['out'] for i in range(N_CORES)])
    return full, res


def kernel(**inputs):
    full, _ = run_raw(inputs)
    return full
